# revision 13
# baseline (speedup 1.0000x reference)
"""L1-attention kernel for Trainium2 (8 NeuronCores).

attn[b, i, j, h] = -(1/sqrt(W)) * sum_w |q[b,j,h,w] - k[b,i,h,w]|

Strategy:
  Shard (batch x head-pair) across the 8 cores. Per core, lay q^T out
  as [p=(head_sub,w)=128, j=512] fp16. For each key i the pairwise
  |q - k_i| is one instruction: DVE tensor_scalar(subtract, abs_max, 0)
  in 4x perf mode for most keys, ACT activation(Abs, scale=-1,
  bias=k_i) for a ~19% slice so both engines run in parallel. The PE
  reduces over the (head,w) partition axis with a one-hot stationary
  that routes key (16b+m)'s two head-sums into PSUM rows (2m, 2m+1) of
  bank b — 16 keys accumulate per [32,512] PSUM tile and each
  stationary is reused across 4 banks, so LDWEIGHTS is amortized 4x.
  Evacuation is an ACT copy with the -1/8 scale fused.
"""

import sys

sys.path.insert(0, "/opt/trn_rl_repo")

import numpy as np

BS, N_CTX, N_HEADS, WIDTH = 2, 512, 8, 64
N_CORES = 8
G = 8  # key groups per core
GK = 64  # keys per group
NB = 4  # PSUM banks (sub-tiles) per group
NM = 16  # keys per bank == distinct stationaries
ACT_M = (5, 10, 15)  # m-slots handled by the scalar engine
SCALE = -1.0 / 8.0

_CACHE = {}


def _build():
    if "nc" in _CACHE:
        return _CACHE["nc"]

    import concourse.bacc as bacc
    import concourse.mybir as mybir
    import concourse.tile as tile

    fp16 = mybir.dt.float16
    fp32 = mybir.dt.float32

    nc = bacc.Bacc(
        "TRN2",
        target_bir_lowering=False,
        debug=False,
        enable_asserts=True,
        num_devices=N_CORES,
    )

    qt_d = nc.dram_tensor("qt", [128, N_CTX], fp16, kind="ExternalInput")
    kt_d = nc.dram_tensor("kt", [128, N_CTX], fp32, kind="ExternalInput")
    sqbm_d = nc.dram_tensor("sqbm", [32, N_CTX], fp32, kind="ExternalInput")
    skb_d = nc.dram_tensor("skb", [32, G * NB], fp32, kind="ExternalInput")
    scl_d = nc.dram_tensor("scl", [32, 1], fp32, kind="ExternalInput")
    out_d = nc.dram_tensor("out", [2, N_CTX, N_CTX], fp32, kind="ExternalOutput")

    # one-hot stationaries: stat[c, m, o] = 1 iff o == 2m + c//64 (o < 32)
    stat_np = np.zeros((128, NM, 32), dtype=np.float16)
    c_idx = np.arange(128)
    for m in range(NM):
        stat_np[c_idx, m, 2 * m + c_idx // 64] = 1.0
    stat_d = nc.inline_tensor(stat_np, name="stat")

    with tile.TileContext(nc) as tc:
        with (
            tc.tile_pool(name="const", bufs=1) as constp,
            tc.tile_pool(name="m", bufs=8) as mp,
            tc.tile_pool(name="ps", bufs=2, space="PSUM") as pp,
            tc.tile_pool(name="o", bufs=4) as outp,
        ):
            qt = constp.tile([128, N_CTX], fp16)
            kt = constp.tile([128, N_CTX], fp32)
            stat = constp.tile([128, NM, 32], fp16)
            sqbm = constp.tile([32, N_CTX], fp32)
            skb = constp.tile([32, G * NB], fp32)
            scl = constp.tile([32, 1], fp32)
            nc.sync.dma_start(qt[:], qt_d[:])
            nc.sync.dma_start(kt[:], kt_d[:])
            nc.sync.dma_start(stat[:], stat_d[:])
            nc.sync.dma_start(sqbm[:], sqbm_d[:])
            nc.sync.dma_start(skb[:], skb_d[:])
            nc.sync.dma_start(scl[:], scl_d[:])

            for g in range(G):
                ps = [
                    pp.tile([32, N_CTX], fp32, tag=f"psb{b}", name=f"ps_{g}_{b}")
                    for b in range(NB)
                ]
                for m in range(NM):
                    for b in range(NB):
                        i = g * GK + NM * b + m
                        mt = mp.tile([128, N_CTX], fp16)
                        if m in ACT_M:
                            nc.scalar.activation(
                                mt[:],
                                qt[:],
                                mybir.ActivationFunctionType.Abs,
                                bias=kt[:, i : i + 1],
                                scale=-1.0,
                            )
                        else:
                            nc.vector.tensor_scalar_min(
                                mt[:], qt[:], kt[:, i : i + 1]
                            )
                        nc.tensor.matmul(
                            ps[b][:],
                            stat[:, m, :],
                            mt[:],
                            start=(m == 0),
                            stop=(m == NM - 1),
                        )
                for b in range(NB):
                    col = g * NB + b
                    t = outp.tile([32, N_CTX], fp32, tag="t")
                    nc.scalar.activation(
                        t[:],
                        ps[b][:],
                        mybir.ActivationFunctionType.Identity,
                        bias=skb[:, col : col + 1],
                        scale=scl[:, 0:1],
                    )
                    o = outp.tile([32, N_CTX], fp32, tag="o")
                    nc.vector.tensor_add(o[:], t[:], sqbm[:])
                    i0 = g * GK + NM * b
                    nc.sync.dma_start(
                        out_d[:, i0 : i0 + NM, :].rearrange("h i j -> i h j"),
                        o[:],
                    )

    nc.compile()
    _CACHE["nc"] = nc
    return nc


def _core_inputs(q, k, c):
    b, hp = divmod(c, 4)
    heads = [2 * hp, 2 * hp + 1]
    qh = q[b][:, heads, :].astype(np.float16)  # [512, 2, 64]
    kh = k[b][:, heads, :].astype(np.float16)
    qt = np.ascontiguousarray(qh.transpose(1, 2, 0).reshape(128, N_CTX))
    kt = np.ascontiguousarray(
        kh.transpose(1, 2, 0).reshape(128, N_CTX).astype(np.float32)
    )
    sq = qh.astype(np.float32).sum(-1)  # [512, 2]
    sk = kh.astype(np.float32).sum(-1)  # [512, 2]
    # rows o = 2m + hh of each (g,b) psum tile hold key i = 64g+16b+m, head hh
    sqbm = np.zeros((32, N_CTX), np.float32)
    skb = np.zeros((32, G * NB), np.float32)
    scl = np.empty((32, 1), np.float32)
    for m in range(NM):
        for hh in range(2):
            o = 2 * m + hh
            if m in ACT_M:
                scl[o, 0] = SCALE  # psum holds sum|q-k| directly
            else:
                scl[o, 0] = 0.25  # psum holds sum min(q,k)
                sqbm[o, :] = SCALE * sq[:, hh]
                for g in range(G):
                    for bb in range(NB):
                        i = g * GK + NM * bb + m
                        skb[o, g * NB + bb] = SCALE * sk[i, hh]
    return {"qt": qt, "kt": kt, "sqbm": sqbm, "skb": skb, "scl": scl}


def kernel(q, k, _trace=False):
    from concourse.bass_utils import run_bass_kernel_spmd

    q = np.asarray(q, dtype=np.float32)
    k = np.asarray(k, dtype=np.float32)
    nc = _build()
    in_maps = [_core_inputs(q, k, c) for c in range(N_CORES)]
    res = run_bass_kernel_spmd(nc, in_maps, core_ids=list(range(N_CORES)), trace=_trace)
    _CACHE["last_results"] = res
    attn = np.empty((BS, N_CTX, N_CTX, N_HEADS), np.float32)
    for c in range(N_CORES):
        b, hp = divmod(c, 4)
        o = res.results[c]["out"]
        attn[b, :, :, 2 * hp] = o[0]
        attn[b, :, :, 2 * hp + 1] = o[1]
    return attn
